# revision 3
# baseline (speedup 1.0000x reference)
"""BiMamba Trainium2 kernel: 8-core SPMD, d_inner-sharded (256 ch/core), both
directions on every core.

Pipeline (dn-major selective scan), staged as sequential TileContexts:
  A: in_proj -> xh, z; u = silu(causal_dwconv(xh)); x_proj partials
  AllReduce x_dbl
  B: delta = softplus(..); replicate to (d,n)-rows via indicator matmuls;
     dA = Exp(A_n*delta); h = tensor_tensor_scan(dA, dB*u) along time;
     y = sum_n C*h via col-packed SEL matmuls
  C: out_partial = ((y_f + D u_f) + flip(y_b + D u_b)) * silu(z) @ WoutT
  ReduceScatter -> per-core (256, 2048) shard; host concatenates.
"""
import sys
for p in ('/opt/trn_rl_repo', '/root/.axon_site/_ro/trn_rl_repo'):
    if p not in sys.path:
        sys.path.insert(0, p)

import numpy as np

D_MODEL, D_INNER, NSTATE, DT_RANK, D_CONV = 1024, 2048, 16, 64, 4
B_SZ, L = 2, 2048
NCORES = 8
DLOC = D_INNER // NCORES
NH = DLOC // 128
NG = DLOC // 8
NGH = 16
NS = L // 512
KD = 96

_BUILT = None


def _build():
    import concourse.bass as bass
    import concourse.tile as tile
    import concourse.mybir as mybir
    from concourse import bacc
    import contextlib

    AF = mybir.ActivationFunctionType
    ALU = mybir.AluOpType
    f32 = mybir.dt.float32

    nc = bacc.Bacc("TRN2", target_bir_lowering=False, debug=False,
                   num_devices=NCORES)

    def inp(name, shape):
        return nc.dram_tensor(name, list(shape), f32, kind="ExternalInput").ap()

    xT = inp("xT", (D_MODEL, B_SZ * L))
    wxh_T = inp("wxh_T", (D_MODEL, DLOC))
    wz_T = inp("wz_T", (D_MODEL, DLOC))
    convw = inp("convw", (2, DLOC, D_CONV))
    convb = inp("convb", (2, DLOC))
    xp_T = inp("xp_T", (2, DLOC, KD))
    dt_T = inp("dt_T", (2, DT_RANK, DLOC))
    bias2 = inp("bias2", (2, DLOC))
    acol = inp("acol", (2, NG, 128))
    rep128 = inp("rep128", (NGH, 128, 128))
    rep16 = inp("rep16", (16, 128))
    sel8 = inp("sel8", (128, 8))
    dsk = inp("dsk", (2, DLOC))
    wout_T = inp("wout_T", (DLOC, D_MODEL))

    out_shard = nc.dram_tensor("out_shard", [B_SZ * D_MODEL // NCORES, L], f32,
                               kind="ExternalOutput").ap()

    # raw DRAM intermediates (all cross-context flows cross a full barrier)
    u_dram = nc.dram_tensor("u_dram", [2 * B_SZ * DLOC, L], f32).ap()
    z_dram = nc.dram_tensor("z_dram", [B_SZ * DLOC, L], f32).ap()
    y_dram = nc.dram_tensor("y_dram", [2 * B_SZ * DLOC, L], f32).ap()
    xdbl_in = nc.dram_tensor("xdbl_in", [2 * B_SZ * KD, L], f32).ap()
    xdbl_out = nc.dram_tensor("xdbl_out", [2 * B_SZ * KD, L], f32).ap()
    rs_in = nc.dram_tensor("rs_in", [B_SZ * D_MODEL, L], f32).ap()
    rs_out = nc.dram_tensor("rs_out", [B_SZ * D_MODEL // NCORES, L], f32).ap()

    LP = L + D_CONV - 1

    # =================== Context A: in_proj/conv/x_proj =====================
    with tile.TileContext(nc) as tc, contextlib.ExitStack() as ctx:
        consts = ctx.enter_context(tc.tile_pool(name="constsA", bufs=1))
        xtp = ctx.enter_context(tc.tile_pool(name="xtp", bufs=2))
        padp = ctx.enter_context(tc.tile_pool(name="padp", bufs=1))
        convp = ctx.enter_context(tc.tile_pool(name="convp", bufs=1))
        up = ctx.enter_context(tc.tile_pool(name="up", bufs=2))
        stg = ctx.enter_context(tc.tile_pool(name="stgA", bufs=2))
        pp = ctx.enter_context(tc.tile_pool(name="ppA", bufs=5, space="PSUM"))
        ppx = ctx.enter_context(tc.tile_pool(name="ppx", bufs=2, space="PSUM"))

        wxh_sb, wz_sb = [], []
        for k in range(8):
            t = consts.tile([128, DLOC], f32, tag=f"wxh{k}")
            nc.sync.dma_start(out=t, in_=wxh_T[k * 128:(k + 1) * 128, :])
            wxh_sb.append(t)
            t = consts.tile([128, DLOC], f32, tag=f"wz{k}")
            nc.sync.dma_start(out=t, in_=wz_T[k * 128:(k + 1) * 128, :])
            wz_sb.append(t)
        xp_sb = [[None] * NH for _ in range(2)]
        for dr in range(2):
            for h in range(NH):
                t = consts.tile([128, KD], f32, tag=f"xp{dr}{h}")
                nc.sync.dma_start(out=t, in_=xp_T[dr, h * 128:(h + 1) * 128, :])
                xp_sb[dr][h] = t
        cw_sb, cb_sb = [], []
        for h in range(NH):
            t = consts.tile([128, 2 * D_CONV], f32, tag=f"cw{h}")
            nc.sync.dma_start(
                out=t,
                in_=bass.AP(tensor=convw.tensor,
                            offset=convw.offset + h * 128 * D_CONV,
                            ap=[[D_CONV, 128], [DLOC * D_CONV, 2], [1, D_CONV]]))
            cw_sb.append(t)
            t2 = consts.tile([128, 2], f32, tag=f"cb{h}")
            nc.sync.dma_start(
                out=t2,
                in_=bass.AP(tensor=convb.tensor, offset=convb.offset + h * 128,
                            ap=[[1, 128], [DLOC, 2]]))
            cb_sb.append(t2)

        for b in range(B_SZ):
            xh_pad = []
            for h in range(NH):
                t = padp.tile([128, LP], f32, tag=f"xhp{h}")
                nc.vector.memset(t[:, 0:D_CONV - 1], 0.0)
                xh_pad.append(t)
            for s in range(NS):
                tok0 = b * L + s * 512
                xts = []
                for k in range(8):
                    t = xtp.tile([128, 512], f32, tag=f"xt{k}")
                    nc.sync.dma_start(out=t, in_=xT[k * 128:(k + 1) * 128,
                                                    tok0:tok0 + 512])
                    xts.append(t)
                for h in range(NH):
                    ps = pp.tile([128, 512], f32, tag="ps")
                    for k in range(8):
                        nc.tensor.matmul(ps, wxh_sb[k][:, h * 128:(h + 1) * 128],
                                         xts[k], start=(k == 0), stop=(k == 7))
                    c0 = D_CONV - 1 + s * 512
                    nc.scalar.activation(xh_pad[h][:, c0:c0 + 512], ps, AF.Copy)
                    ps = pp.tile([128, 512], f32, tag="ps")
                    for k in range(8):
                        nc.tensor.matmul(ps, wz_sb[k][:, h * 128:(h + 1) * 128],
                                         xts[k], start=(k == 0), stop=(k == 7))
                    zs512 = stg.tile([128, 512], f32, tag="zs512")
                    nc.scalar.activation(zs512, ps, AF.Copy)
                    nc.sync.dma_start(
                        out=z_dram[(b * NH + h) * 128:(b * NH + h + 1) * 128,
                                   s * 512:s * 512 + 512],
                        in_=zs512)
            for dr in range(2):
                us = [None] * NH
                for h in range(NH):
                    if dr == 0:
                        pad = xh_pad[h]
                    else:
                        pad = padp.tile([128, LP], f32, tag=f"xhpb{h}")
                        nc.vector.memset(pad[:, 0:D_CONV - 1], 0.0)
                        nc.vector.tensor_copy(
                            pad[:, D_CONV - 1:LP],
                            xh_pad[h][:, D_CONV - 1:LP][:, ::-1])
                    c0 = convp.tile([128, L], f32, tag="c0")
                    nc.scalar.activation(c0, pad[:, D_CONV - 1:LP], AF.Identity,
                                         scale=cw_sb[h][:, dr * 4 + 3:dr * 4 + 4],
                                         bias=cb_sb[h][:, dr:dr + 1])
                    for k in (2, 1, 0):
                        c1 = convp.tile([128, L], f32, tag=f"c{(k % 2) + 1}")
                        nc.vector.scalar_tensor_tensor(
                            c1, pad[:, k:k + L],
                            cw_sb[h][:, dr * 4 + k:dr * 4 + k + 1],
                            c0, ALU.mult, ALU.add)
                        c0 = c1
                    # silu
                    e1 = convp.tile([128, L], f32, tag="sle")
                    nc.scalar.activation(e1, c0, AF.Exp)
                    sp = convp.tile([128, L], f32, tag="sls")
                    nc.scalar.activation(sp, e1, AF.Ln, bias=1.0)
                    am = convp.tile([128, L], f32, tag="sle")
                    nc.vector.tensor_sub(am, c0, sp)
                    sg = convp.tile([128, L], f32, tag="sls")
                    nc.scalar.activation(sg, am, AF.Exp)
                    ut = up.tile([128, L], f32, tag=f"u{h}")
                    nc.vector.tensor_mul(ut, c0, sg)
                    us[h] = ut
                    nc.sync.dma_start(
                        out=u_dram[((dr * B_SZ + b) * NH + h) * 128:
                                   ((dr * B_SZ + b) * NH + h + 1) * 128, :],
                        in_=ut)
                q = dr * B_SZ + b
                for s in range(NS):
                    ps = ppx.tile([KD, 512], f32, tag="psx")
                    for h in range(NH):
                        nc.tensor.matmul(ps, xp_sb[dr][h],
                                         us[h][:, s * 512:s * 512 + 512],
                                         start=(h == 0), stop=(h == NH - 1))
                    xd512 = stg.tile([KD, 512], f32, tag="xd512")
                    nc.scalar.activation(xd512, ps, AF.Copy)
                    nc.sync.dma_start(
                        out=xdbl_in[q * KD:(q + 1) * KD, s * 512:s * 512 + 512],
                        in_=xd512)

    # =================== Context: AllReduce =================================
    with tile.TileContext(nc) as tc:
        nc.gpsimd.collective_compute(
            "AllReduce", mybir.AluOpType.add,
            replica_groups=[list(range(NCORES))],
            ins=[xdbl_in], outs=[xdbl_out])

    # =================== Context B: scan ====================================
    with tile.TileContext(nc) as tc, contextlib.ExitStack() as ctx:
        consts = ctx.enter_context(tc.tile_pool(name="constsB", bufs=1))
        bp = ctx.enter_context(tc.tile_pool(name="bp", bufs=1))
        slp = ctx.enter_context(tc.tile_pool(name="slp", bufs=2))
        scanp = ctx.enter_context(tc.tile_pool(name="scanp", bufs=4))
        hp = ctx.enter_context(tc.tile_pool(name="hp", bufs=2))
        cyp = ctx.enter_context(tc.tile_pool(name="cyp", bufs=4))
        yp = ctx.enter_context(tc.tile_pool(name="yp", bufs=2))
        pp = ctx.enter_context(tc.tile_pool(name="ppB", bufs=6, space="PSUM"))
        ppy = ctx.enter_context(tc.tile_pool(name="ppy", bufs=2, space="PSUM"))

        dt_sb = []
        for dr in range(2):
            t = consts.tile([DT_RANK, DLOC], f32, tag=f"dt{dr}")
            nc.sync.dma_start(out=t, in_=dt_T[dr])
            dt_sb.append(t)
        rep128_sb = []
        for gh in range(NGH):
            t = consts.tile([128, 128], f32, tag=f"rep{gh}")
            nc.sync.dma_start(out=t, in_=rep128[gh])
            rep128_sb.append(t)
        rep16_sb = consts.tile([16, 128], f32, tag="rep16")
        nc.sync.dma_start(out=rep16_sb, in_=rep16)
        sel8_sb = consts.tile([128, 8], f32, tag="sel8")
        nc.sync.dma_start(out=sel8_sb, in_=sel8)
        acol_sb = consts.tile([128, 2 * NG], f32, tag="acol")
        nc.sync.dma_start(
            out=acol_sb,
            in_=bass.AP(tensor=acol.tensor, offset=acol.offset,
                        ap=[[1, 128], [128, 2 * NG]]))
        b2_sb = []
        for h in range(NH):
            t2 = consts.tile([128, 2], f32, tag=f"b2{h}")
            nc.sync.dma_start(
                out=t2,
                in_=bass.AP(tensor=bias2.tensor, offset=bias2.offset + h * 128,
                            ap=[[1, 128], [DLOC, 2]]))
            b2_sb.append(t2)

        for dr in range(2):
            for b in range(B_SZ):
                q = dr * B_SZ + b
                dtlr = bp.tile([DT_RANK, L], f32, tag="dtlr")
                nc.sync.dma_start(out=dtlr, in_=xdbl_out[q * KD:q * KD + 64, :])
                Bt = bp.tile([16, L], f32, tag="Bt")
                nc.sync.dma_start(out=Bt, in_=xdbl_out[q * KD + 64:q * KD + 80, :])
                Ct = bp.tile([16, L], f32, tag="Ct")
                nc.sync.dma_start(out=Ct, in_=xdbl_out[q * KD + 80:q * KD + 96, :])
                Brep = bp.tile([128, L], f32, tag="Brep")
                Crep = bp.tile([128, L], f32, tag="Crep")
                for s in range(NS):
                    sl = slice(s * 512, s * 512 + 512)
                    ps = pp.tile([128, 512], f32, tag="ps")
                    nc.tensor.matmul(ps, rep16_sb, Bt[:, sl], start=True, stop=True)
                    nc.scalar.activation(Brep[:, sl], ps, AF.Copy)
                    ps = pp.tile([128, 512], f32, tag="ps")
                    nc.tensor.matmul(ps, rep16_sb, Ct[:, sl], start=True, stop=True)
                    nc.scalar.activation(Crep[:, sl], ps, AF.Copy)
                dl, du = [None] * NH, [None] * NH
                for h in range(NH):
                    dlt = bp.tile([128, L], f32, tag=f"dl{h}")
                    for s in range(NS):
                        sl = slice(s * 512, s * 512 + 512)
                        ps = pp.tile([128, 512], f32, tag="ps")
                        nc.tensor.matmul(ps, dt_sb[dr][:, h * 128:(h + 1) * 128],
                                         dtlr[:, sl], start=True, stop=True)
                        e512 = slp.tile([128, 512], f32, tag="e512")
                        nc.scalar.activation(e512, ps, AF.Exp,
                                             bias=b2_sb[h][:, dr:dr + 1])
                        nc.scalar.activation(dlt[:, sl], e512, AF.Ln, bias=1.0)
                    dl[h] = dlt
                    dut = bp.tile([128, L], f32, tag=f"du{h}")
                    for s in range(NS):
                        sl = slice(s * 512, s * 512 + 512)
                        uslc = slp.tile([128, 512], f32, tag="uslc")
                        nc.sync.dma_start(
                            out=uslc,
                            in_=u_dram[(q * NH + h) * 128:(q * NH + h + 1) * 128, sl])
                        nc.vector.tensor_mul(dut[:, sl], dlt[:, sl], uslc)
                    du[h] = dut
                for sup in range(8):
                    hprev = [None] * 4
                    ypk = yp.tile([128, L], f32, tag="ypk")
                    for s in range(NS):
                        sl = slice(s * 512, s * 512 + 512)
                        py = ppy.tile([128, 512], f32, tag="psY")
                        for j in range(4):
                            g = sup * 4 + j
                            h = g // NGH
                            gh = g % NGH
                            ps = pp.tile([128, 512], f32, tag="ps")
                            nc.tensor.matmul(ps, rep128_sb[gh], dl[h][:, sl],
                                             start=True, stop=True)
                            dA = scanp.tile([128, 512], f32, tag="dA")
                            nc.scalar.activation(
                                dA, ps, AF.Exp,
                                scale=acol_sb[:, dr * NG + g:dr * NG + g + 1])
                            ps = pp.tile([128, 512], f32, tag="ps")
                            nc.tensor.matmul(ps, rep128_sb[gh], du[h][:, sl],
                                             start=True, stop=True)
                            dBu = scanp.tile([128, 512], f32, tag="dBu")
                            nc.vector.tensor_mul(dBu, ps, Brep[:, sl])
                            ht = hp.tile([128, 512], f32, tag=f"h{j}")
                            init = 0.0 if s == 0 else hprev[j][:, 511:512]
                            nc.vector.tensor_tensor_scan(ht, dA, dBu, init,
                                                         ALU.mult, ALU.add)
                            hprev[j] = ht
                            cy = cyp.tile([128, 512], f32, tag="cy")
                            nc.vector.tensor_mul(cy, ht, Crep[:, sl])
                            nc.tensor.matmul(py[32 * j:32 * j + 8, :], sel8_sb,
                                             cy, start=True, stop=True,
                                             tile_position=(0, 32 * j),
                                             skip_group_check=True)
                        nc.scalar.activation(ypk[:, sl], py, AF.Copy)
                    for j in range(4):
                        ch0 = (sup * 4 + j) * 8
                        nc.sync.dma_start(
                            out=y_dram[q * DLOC + ch0:q * DLOC + ch0 + 8, :],
                            in_=ypk[32 * j:32 * j + 8, :])

    # =================== Context C: gate + out_proj =========================
    with tile.TileContext(nc) as tc, contextlib.ExitStack() as ctx:
        consts = ctx.enter_context(tc.tile_pool(name="constsC", bufs=1))
        stg = ctx.enter_context(tc.tile_pool(name="stgC", bufs=1))
        gp = ctx.enter_context(tc.tile_pool(name="gp", bufs=1))
        pp = ctx.enter_context(tc.tile_pool(name="ppC", bufs=6, space="PSUM"))

        wout_sb = []
        for h in range(NH):
            t = consts.tile([128, D_MODEL], f32, tag=f"wout{h}")
            nc.sync.dma_start(out=t, in_=wout_T[h * 128:(h + 1) * 128, :])
            wout_sb.append(t)
        dsk_sb = []
        for h in range(NH):
            t2 = consts.tile([128, 2], f32, tag=f"dk{h}")
            nc.sync.dma_start(
                out=t2,
                in_=bass.AP(tensor=dsk.tensor, offset=dsk.offset + h * 128,
                            ap=[[1, 128], [DLOC, 2]]))
            dsk_sb.append(t2)

        for b in range(B_SZ):
            gf = [None] * NH
            for h in range(NH):
                yf = stg.tile([128, L], f32, tag="yf")
                nc.sync.dma_start(
                    out=yf, in_=y_dram[(b * NH + h) * 128:(b * NH + h + 1) * 128, :])
                uf = stg.tile([128, L], f32, tag="uf")
                nc.sync.dma_start(
                    out=uf, in_=u_dram[(b * NH + h) * 128:(b * NH + h + 1) * 128, :])
                t1 = stg.tile([128, L], f32, tag="t1")
                nc.vector.scalar_tensor_tensor(t1, uf, dsk_sb[h][:, 0:1], yf,
                                               ALU.mult, ALU.add)
                qb = B_SZ + b
                yb = stg.tile([128, L], f32, tag="yb")
                nc.sync.dma_start(
                    out=yb, in_=y_dram[(qb * NH + h) * 128:(qb * NH + h + 1) * 128, :])
                ub = stg.tile([128, L], f32, tag="ub2")
                nc.sync.dma_start(
                    out=ub, in_=u_dram[(qb * NH + h) * 128:(qb * NH + h + 1) * 128, :])
                t2 = stg.tile([128, L], f32, tag="t2")
                nc.vector.scalar_tensor_tensor(t2, ub, dsk_sb[h][:, 1:2], yb,
                                               ALU.mult, ALU.add)
                comb = stg.tile([128, L], f32, tag="comb")
                nc.vector.tensor_add(comb, t1, t2[:, ::-1])
                zt = stg.tile([128, L], f32, tag="zt")
                nc.sync.dma_start(
                    out=zt, in_=z_dram[(b * NH + h) * 128:(b * NH + h + 1) * 128, :])
                e1 = stg.tile([128, L], f32, tag="zse")
                nc.scalar.activation(e1, zt, AF.Exp)
                sp = stg.tile([128, L], f32, tag="zsp")
                nc.scalar.activation(sp, e1, AF.Ln, bias=1.0)
                am = stg.tile([128, L], f32, tag="zse")
                nc.vector.tensor_sub(am, zt, sp)
                sg = stg.tile([128, L], f32, tag="zsp")
                nc.scalar.activation(sg, am, AF.Exp)
                zs = stg.tile([128, L], f32, tag="zss")
                nc.vector.tensor_mul(zs, zt, sg)
                gt = gp.tile([128, L], f32, tag=f"g{h}")
                nc.vector.tensor_mul(gt, comb, zs)
                gf[h] = gt
            for o in range(8):
                ost = stg.tile([128, L], f32, tag="ost")
                for s in range(NS):
                    sl = slice(s * 512, s * 512 + 512)
                    ps = pp.tile([128, 512], f32, tag="ps")
                    for h in range(NH):
                        nc.tensor.matmul(ps, wout_sb[h][:, o * 128:(o + 1) * 128],
                                         gf[h][:, sl], start=(h == 0),
                                         stop=(h == NH - 1))
                    nc.scalar.activation(ost[:, sl], ps, AF.Copy)
                nc.sync.dma_start(
                    out=rs_in[b * D_MODEL + o * 128:b * D_MODEL + (o + 1) * 128, :],
                    in_=ost)

    # =================== Context: ReduceScatter =============================
    with tile.TileContext(nc) as tc:
        nc.gpsimd.collective_compute(
            "ReduceScatter", mybir.AluOpType.add,
            replica_groups=[list(range(NCORES))],
            ins=[rs_in], outs=[rs_out])

    # =================== Context: final copy ================================
    with tile.TileContext(nc) as tc, contextlib.ExitStack() as ctx:
        stg = ctx.enter_context(tc.tile_pool(name="fin", bufs=2))
        for r in range(B_SZ * D_MODEL // NCORES // 128):
            fin = stg.tile([128, L], f32, tag="fin")
            nc.sync.dma_start(out=fin, in_=rs_out[r * 128:(r + 1) * 128, :])
            nc.sync.dma_start(out=out_shard[r * 128:(r + 1) * 128, :], in_=fin)

    nc.compile()
    return nc


def _prep_in_maps(inputs):
    x = np.ascontiguousarray(inputs["x"], np.float32)
    xT = np.ascontiguousarray(x.reshape(B_SZ * L, D_MODEL).T)

    rep128 = np.zeros((NGH, 128, 128), np.float32)
    for gh in range(NGH):
        for d in range(8):
            rep128[gh, gh * 8 + d, 16 * d:16 * d + 16] = 1.0
    rep16 = np.zeros((16, 128), np.float32)
    sel8 = np.zeros((128, 8), np.float32)
    for d in range(8):
        for n in range(16):
            rep16[n, 16 * d + n] = 1.0
            sel8[16 * d + n, d] = 1.0

    A_f = -np.exp(np.asarray(inputs["A_log"], np.float32))
    A_b = -np.exp(np.asarray(inputs["A_b_log"], np.float32))

    in_maps = []
    for c in range(NCORES):
        sl = slice(c * DLOC, (c + 1) * DLOC)
        acol = np.zeros((2, NG, 128), np.float32)
        for dr, A in enumerate((A_f, A_b)):
            Asl = A[sl]
            for g in range(NG):
                for d in range(8):
                    acol[dr, g, 16 * d:16 * d + 16] = Asl[g * 8 + d]
        m = {
            "xT": xT,
            "wxh_T": np.ascontiguousarray(np.asarray(inputs["in_proj_w"])[sl].T),
            "wz_T": np.ascontiguousarray(
                np.asarray(inputs["in_proj_w"])[D_INNER + c * DLOC:
                                                D_INNER + (c + 1) * DLOC].T),
            "convw": np.stack([np.asarray(inputs["conv_w"])[sl, 0, :],
                               np.asarray(inputs["conv_w_b"])[sl, 0, :]]),
            "convb": np.stack([np.asarray(inputs["conv_bias"])[sl],
                               np.asarray(inputs["conv_bias_b"])[sl]]),
            "xp_T": np.stack([np.asarray(inputs["x_proj_w"])[:, sl].T,
                              np.asarray(inputs["x_proj_b_w"])[:, sl].T]),
            "dt_T": np.stack([np.asarray(inputs["dt_proj_w"])[sl].T,
                              np.asarray(inputs["dt_proj_b_w"])[sl].T]),
            "bias2": np.stack([2.0 * np.asarray(inputs["dt_proj_bias"])[sl],
                               2.0 * np.asarray(inputs["dt_proj_b_bias"])[sl]]),
            "acol": acol,
            "rep128": rep128,
            "rep16": rep16,
            "sel8": sel8,
            "dsk": np.stack([np.asarray(inputs["D"])[sl],
                             np.asarray(inputs["D_b"])[sl]]),
            "wout_T": np.ascontiguousarray(np.asarray(inputs["out_proj_w"])[:, sl].T),
        }
        m = {k: np.ascontiguousarray(v, np.float32) for k, v in m.items()}
        in_maps.append(m)
    return in_maps


def get_built():
    global _BUILT
    if _BUILT is None:
        _BUILT = _build()
    return _BUILT


def assemble(results):
    full = np.concatenate([results[c]["out_shard"] for c in range(NCORES)], axis=0)
    return np.ascontiguousarray(
        full.reshape(B_SZ, D_MODEL, L).transpose(0, 2, 1))


def kernel(**inputs):
    from concourse.bass_utils import run_bass_kernel_spmd
    nc = get_built()
    in_maps = _prep_in_maps(inputs)
    res = run_bass_kernel_spmd(nc, in_maps, list(range(NCORES)))
    return assemble(res.results)
